# revision 26
# baseline (speedup 1.0000x reference)
"""Bass/Tile kernel for a single attention head, data-parallel over B=8 on
8 TRN2 NeuronCores (one batch element per core, no collectives).

Per-core problem (S=2048, D=1024, H=128):
    q = Xq @ Wq + bq ; k = Xk @ Wk + bk ; v = Xv @ Wv + bv
    out = softmax(q k^T / sqrt(H)) v

v5 design notes (PE contracts over the partition dim):
  - X^T built on the HOST (numpy transpose + bf16 cast + repack) so the
    PE spends zero cycles transposing inputs; all DMA lines are 2-8KB.
  - Every matmul pays ~LDWEIGHTS(stat cols) + N + fixed overhead, so the
    structure minimizes instruction count and maximizes N: projections
    and scores use N=512 (the PSUM-bank max for f32), k/q/v stream in
    quarters.
  - scoresT [j, i] per j-tile; exp((1/sqrt H)x) is one ACT op per
    (j-tile, i-half) PSUM->SBUF bf16.  The ACT stream (~43us) is one of
    two walls; the schedule starts it ASAP (byte-priority q half0 + k
    first) and never lets it starve (q2/q3 projections are emitted
    INSIDE the k loop; PE is in-order).
  - v projected to natural [s, h] with NO bias: since softmax rows sum
    to 1, out = num/den + bv exactly, so bv folds into the final
    normalization (scalar_tensor_tensor: (acc*rc) + bv) for free.
  - AV keeps the fused form: stationary exp^T slice [j, i-tile], moving
    v|ones [j, 129] -> numerator AND row-sums in one accumulation.
    3 i-tiles per PSUM bank; DVE drains move 3 tiles per op.  The upper
    i-half runs in j-QUARTER phases chasing the exp i1 stream so only
    ~2us of AV trails the last exp; the lower i-half (needs only early
    i0 exp + v) fills PE slack during the exp stream.
  - Output leaves as [p, itile, h] packed halves (4KB DMA lines), host
    unpacks.  Load doorbells: weights on GpSimd queue, X on Sync
    (each dma_start costs ~680ns of issue time on its queue).
"""

import sys

if "/opt/trn_rl_repo" not in sys.path:
    sys.path.insert(0, "/opt/trn_rl_repo")

import numpy as np

import concourse.bass as bass
import concourse.tile as tile
from concourse import bacc, mybir
from concourse.bass_utils import run_bass_kernel_spmd

P = 128          # partitions
S = 2048         # sequence length (per core)
D = 1024         # input dim
H = 128          # head dim (Dq = Dk)
ST = S // P      # 16 s-tiles
DC = D // P      # 8 d-chunks
NQ = 4           # s-quarters
QS = S // NQ     # 512
N_CORES = 8

F32 = mybir.dt.float32
BF16 = mybir.dt.bfloat16
FP8 = mybir.dt.float8e4
W_SCALE = 16.0
AF = mybir.ActivationFunctionType

SOFTMAX_SCALE = 1.0 / float(np.sqrt(H))


def _build_kernel(tc, ins, out_ap):
    nc = tc.nc
    (qp, kp, vp, wq_ap, bq_ap, wk_ap, bk_ap, wv_ap, bv_ap) = ins

    with (
        tc.tile_pool(name="consts", bufs=1) as consts,
        tc.tile_pool(name="proj", bufs=1) as projp,
        tc.tile_pool(name="expp", bufs=1) as expp,
        tc.tile_pool(name="vext", bufs=1) as vexp,
        tc.tile_pool(name="accp", bufs=1) as accp,
        tc.tile_pool(name="outp", bufs=1) as outp,
        tc.tile_pool(name="xq", bufs=4) as xqp,
        tc.tile_pool(name="xk", bufs=4) as xkp,
        tc.tile_pool(name="xv", bufs=4) as xvp,
    ):
        # ---- tiny consts (no DMA) ----
        warm_a = consts.tile([P, P], BF16, tag="warm_a")
        nc.gpsimd.memset(warm_a, 0.5)
        warm_sink = nc.dram_tensor("warm_sink", [P, P], F32)

        # ---- load doorbells: weights/biases on GpSimd, X on Sync ----
        wq = consts.tile([P, DC, H], FP8, tag="wq")
        nc.gpsimd.dma_start(out=wq, in_=wq_ap)
        bq = consts.tile([P, 1], F32, tag="bq")
        nc.gpsimd.dma_start(out=bq, in_=bq_ap)
        wk = consts.tile([P, DC, H], FP8, tag="wk")
        nc.gpsimd.dma_start(out=wk, in_=wk_ap)
        bk = consts.tile([P, 1], F32, tag="bk")
        nc.gpsimd.dma_start(out=bk, in_=bk_ap)

        xq_tiles = [
            xqp.tile([P, DC, QS], FP8, tag="xq", name=f"xq{nq}")
            for nq in range(NQ)
        ]
        xk_tiles = [
            xkp.tile([P, DC, QS], FP8, tag="xk", name=f"xk{t}")
            for t in range(NQ)
        ]
        xv_tiles = [
            xvp.tile([P, DC, QS], BF16, tag="xv", name=f"xv{nq}")
            for nq in range(NQ)
        ]
        # byte-priority: q half0 + k feed the exp stream, v is last
        nc.sync.dma_start(out=xq_tiles[0], in_=qp[0])
        nc.sync.dma_start(out=xk_tiles[0], in_=kp[0])
        nc.sync.dma_start(out=xq_tiles[1], in_=qp[1])
        for t in range(1, NQ):
            nc.sync.dma_start(out=xk_tiles[t], in_=kp[t])
        nc.sync.dma_start(out=xq_tiles[2], in_=qp[2])
        nc.sync.dma_start(out=xq_tiles[3], in_=qp[3])
        for t in range(NQ):
            nc.sync.dma_start(out=xv_tiles[t], in_=vp[t])

        # wv/bvr are needed late; their doorbells ride AFTER the
        # critical q/k prefix so their bytes don't compete with it
        wv = consts.tile([P, DC, H], BF16, tag="wv")
        nc.gpsimd.dma_start(out=wv, in_=wv_ap)
        bvr = consts.tile([P, H], F32, tag="bvr")
        nc.gpsimd.dma_start(out=bvr, in_=bv_ap)
        # preload the ACT exp table set (~2.7us) during DMA dead time
        dummy = consts.tile([P, 1], F32, tag="dummy")
        nc.gpsimd.memset(dummy, 0.0)
        exp_sink = consts.tile([P, 1], BF16, tag="exp_sink")
        nc.scalar.activation(exp_sink, dummy, AF.Exp, bias=0.0, scale=1.0)

        # ---- persistent SBUF tiles ----
        qTq = [
            projp.tile([P, QS], BF16, tag=f"qT{i}", name=f"qT{i}")
            for i in range(NQ)
        ]
        kTq = [
            projp.tile([P, QS], BF16, tag=f"kT{i}", name=f"kT{i}")
            for i in range(NQ)
        ]
        ex = [
            [
                expp.tile([P, 4, 1024], BF16, tag=f"ex{h}{jq}", name=f"ex{h}{jq}")
                for jq in range(NQ)
            ]
            for h in range(2)
        ]
        vx = [
            vexp.tile([P, 4, H + 1], BF16, tag=f"vx{jq}", name=f"vx{jq}")
            for jq in range(NQ)
        ]
        for jq in range(NQ):
            nc.gpsimd.memset(vx[jq][:, :, H : H + 1], 1.0)
        acc = accp.tile([P, ST, H + 4], F32, tag="acc")
        rc_all = accp.tile([P, ST], F32, tag="rc_all")
        out_sb = [
            outp.tile([P, 8, H], BF16, tag=f"osb{hf}", name=f"osb{hf}")
            for hf in range(2)
        ]

        with (
            tc.tile_pool(name="psS", bufs=2, space="PSUM") as psS,   # 2x2 banks
            tc.tile_pool(name="psP", bufs=2, space="PSUM") as psP,   # 2x1 banks
            tc.tile_pool(name="psB", bufs=2, space="PSUM") as psB,   # 2x1 banks
        ):
            # ---- PE warm-up (HAM clock ramp needs sustained activity) ----
            ps_w = psP.tile([P, QS], F32, tag="pp", name="ps_w")
            for _ in range(90):
                nc.tensor.matmul(
                    ps_w[:, 0:P], warm_a, warm_a, start=True, stop=True
                )
            warm_sb = consts.tile([P, P], F32, tag="warm_sb")
            nc.vector.tensor_copy(warm_sb, ps_w[:, 0:P])
            nc.sync.dma_start(out=warm_sink[:, :], in_=warm_sb)

            def pe_keepalive(n):
                for _ in range(n):
                    nc.tensor.matmul(
                        ps_w[:, 0:P], warm_a, warm_a, start=True, stop=True
                    )

            def proj_quarter(xt, w, b, dst):
                ps = psP.tile([P, QS], F32, tag="pp")
                for dc in range(DC):
                    nc.tensor.matmul(
                        ps,
                        w[:, dc, :],
                        xt[:, dc, :],
                        start=(dc == 0),
                        stop=(dc == DC - 1),
                    )
                # fp8 weights ride x16 scaled (half of W is e4m3-subnormal
                # otherwise); fold the 1/16 into the bias drain
                nc.vector.tensor_scalar(
                    dst, ps, 1.0 / W_SCALE, b,
                    mybir.AluOpType.mult, mybir.AluOpType.add,
                )

            def scores_exp_split(jt):
                """scores+exp for (jt, i0) in two i-quarter ACT ops so
                the stream starts before q quarter 1 has even arrived."""
                kt_sl = kTq[0][:, (jt % 4) * P : (jt % 4 + 1) * P]
                pss = psS.tile([P, 1024], F32, tag="ps", name=f"pss_sp{jt}")
                nc.tensor.matmul(
                    pss[:, 0:QS], kt_sl, qTq[0], start=True, stop=True
                )
                nc.scalar.activation(
                    ex[0][0][:, jt, 0:QS],
                    pss[:, 0:QS],
                    AF.Exp,
                    bias=0.0,
                    scale=SOFTMAX_SCALE,
                )
                return pss

            def scores_exp_split2(jt, pss):
                nc.tensor.matmul(
                    pss[:, QS:1024], kTq[0][:, (jt % 4) * P : (jt % 4 + 1) * P],
                    qTq[1], start=True, stop=True
                )
                nc.scalar.activation(
                    ex[0][0][:, jt, QS:1024],
                    pss[:, QS:1024],
                    AF.Exp,
                    bias=0.0,
                    scale=SOFTMAX_SCALE,
                )

            def scores_exp(jt, hf):
                kt_sl = kTq[jt // 4][:, (jt % 4) * P : (jt % 4 + 1) * P]
                pss = psS.tile([P, 1024], F32, tag="ps")
                for nb in range(2):
                    nc.tensor.matmul(
                        pss[:, nb * QS : (nb + 1) * QS],
                        kt_sl,
                        qTq[2 * hf + nb],
                        start=True,
                        stop=True,
                    )
                nc.scalar.activation(
                    ex[hf][jt // 4][:, jt % 4, :],
                    pss,
                    AF.Exp,
                    bias=0.0,
                    scale=SOFTMAX_SCALE,
                )

            # v quarter projection split into s-tile emission chunks so
            # it can fill PE slack between ACT-paced scores
            vps = {}

            def vproj_stile(jq, st):
                if jq not in vps:
                    vps[jq] = psP.tile([P, QS], F32, tag="pp", name=f"vps{jq}")
                ps = vps[jq]
                for dc in range(DC):
                    nc.tensor.matmul(
                        ps[:, st * P : (st + 1) * P],
                        xv_tiles[jq][:, dc, st * P : (st + 1) * P],
                        wv[:, dc, :],
                        start=(dc == 0),
                        stop=(dc == DC - 1),
                    )

            def vdrain(jq):
                nc.vector.tensor_copy(
                    vx[jq][:, :, 0:H],
                    vps[jq].rearrange("p (a b) -> p a b", b=P),
                )

            GROUPS = [(0, 3), (3, 3), (6, 2)]

            def av_group(ihalf, j0, nj, g0, glen, first):
                """AV partials: i-tiles [8ihalf+g0, +glen) x j-tiles
                [j0, j0+nj), 3 i-tiles per PSUM bank, one DVE drain."""
                i0 = 8 * ihalf
                po = psB.tile([P, 3, H + 4], F32, tag="po")
                for m in range(glen):
                    k = g0 + m
                    for dj in range(nj):
                        jt = j0 + dj
                        nc.tensor.matmul(
                            po[:, m, 0 : H + 1],
                            ex[ihalf][jt // 4][:, jt % 4, k * P : (k + 1) * P],
                            vx[jt // 4][:, jt % 4, :],
                            start=(dj == 0),
                            stop=(dj == nj - 1),
                        )
                dst = acc[:, i0 + g0 : i0 + g0 + glen, :]
                src = po[:, 0:glen, :]
                if first:
                    nc.vector.tensor_copy(dst, src)
                else:
                    nc.vector.tensor_add(dst, dst, src)

            def norm_store_pipelined():
                """Only j14-15 trails the final exp; per group: DVE
                drain-add + batched recip, scale-muls on the (now idle)
                ACT engine, bv-add + store on DVE."""
                for g0, glen in GROUPS:
                    av_group(1, 14, 2, g0, glen, False)
                    a0 = 8 + g0
                    nc.vector.reciprocal(
                        rc_all[:, a0 : a0 + glen],
                        acc[:, a0 : a0 + glen, H : H + 1].squeeze(-1),
                    )
                    dst = out_sb[1][:, g0 : g0 + glen, :]
                    for m in range(glen):
                        nc.scalar.activation(
                            out_sb[1][:, g0 + m, :],
                            acc[:, a0 + m, 0:H],
                            AF.Copy,
                            bias=0.0,
                            scale=rc_all[:, a0 + m : a0 + m + 1],
                        )
                    bv_bc = bvr[:, :].unsqueeze(1).broadcast_to([P, glen, H])
                    nc.vector.tensor_add(dst, dst, bv_bc)
                    nc.sync.dma_start(
                        out=out_ap[1, :, g0 : g0 + glen, :], in_=dst
                    )

            def norm_store(ihalf):
                """Batched reciprocal; out = acc*rc + bv (bv folds in
                free since softmax rows sum to 1); one packed half DMA."""
                i0 = 8 * ihalf
                nc.vector.reciprocal(
                    rc_all[:, i0 : i0 + 8],
                    acc[:, i0 : i0 + 8, H : H + 1].squeeze(-1),
                )
                for g0, glen in GROUPS:
                    rc_bc = (
                        rc_all[:, i0 + g0 : i0 + g0 + glen]
                        .unsqueeze(-1)
                        .broadcast_to([P, glen, H])
                    )
                    dst = out_sb[ihalf][:, g0 : g0 + glen, :]
                    nc.vector.tensor_mul(
                        dst, acc[:, i0 + g0 : i0 + g0 + glen, 0:H], rc_bc
                    )
                    bv_bc = bvr[:, :].unsqueeze(1).broadcast_to(
                        [P, glen, H]
                    )
                    nc.vector.tensor_add(dst, dst, bv_bc)
                    nc.sync.dma_start(
                        out=out_ap[ihalf, :, g0 : g0 + glen, :], in_=dst
                    )

            # ---- emission order == intended engine execution order ----
            # PE is in-order: every insertion is placed at the point
            # where its data has just arrived, sized ~<=2us so the
            # ACT-paced scores stream never starves for long.
            proj_quarter(xq_tiles[0], wq, bq, qTq[0])
            pe_keepalive(20)
            proj_quarter(xk_tiles[0], wk, bk, kTq[0])
            ps_j0 = scores_exp_split(0)
            ps_j1 = scores_exp_split(1)
            proj_quarter(xq_tiles[1], wq, bq, qTq[1])
            scores_exp_split2(0, ps_j0)
            scores_exp_split2(1, ps_j1)
            ps_j2 = scores_exp_split(2)
            scores_exp_split2(2, ps_j2)
            ps_j3 = scores_exp_split(3)
            scores_exp_split2(3, ps_j3)
            for kq in range(1, NQ):
                proj_quarter(xk_tiles[kq], wk, bk, kTq[kq])
                for jt in range(4 * kq, 4 * kq + 4):
                    scores_exp(jt, 0)
                    if jt == 12:
                        vproj_stile(0, 0)
                        vproj_stile(0, 1)
                    elif jt == 13:
                        proj_quarter(xq_tiles[2], wq, bq, qTq[2])
                    elif jt == 14:
                        vproj_stile(0, 2)
                        vproj_stile(0, 3)
                        vdrain(0)
                        av_group(0, 0, 4, 0, 3, True)
                    elif jt == 15:
                        proj_quarter(xq_tiles[3], wq, bq, qTq[3])
                        av_group(0, 0, 4, 3, 3, True)

            for jt in range(ST):
                scores_exp(jt, 1)
                if jt == 0:
                    av_group(0, 0, 4, 6, 2, True)
                    vproj_stile(1, 0)
                    vproj_stile(1, 1)
                elif jt == 1:
                    vproj_stile(1, 2)
                    vproj_stile(1, 3)
                    vdrain(1)
                elif jt == 2:
                    vproj_stile(2, 0)
                    vproj_stile(2, 1)
                    vproj_stile(2, 2)
                    vproj_stile(2, 3)
                    vdrain(2)
                elif jt == 3:
                    av_group(0, 4, 4, 0, 3, False)
                elif jt == 4:
                    av_group(0, 4, 4, 3, 3, False)
                elif jt == 5:
                    av_group(0, 4, 4, 6, 2, False)
                elif jt == 6:
                    vproj_stile(3, 0)
                    vproj_stile(3, 1)
                elif jt == 7:
                    vproj_stile(3, 2)
                    vproj_stile(3, 3)
                    vdrain(3)
                elif jt == 8:
                    av_group(0, 8, 8, 0, 3, False)
                elif jt == 9:
                    av_group(0, 8, 8, 3, 3, False)
                elif jt == 10:
                    av_group(0, 8, 8, 6, 2, False)
                elif jt == 11:
                    av_group(1, 0, 4, 0, 3, True)
                elif jt == 12:
                    av_group(1, 0, 4, 3, 3, True)
                elif jt == 13:
                    av_group(1, 0, 4, 6, 2, True)
                    av_group(1, 4, 4, 0, 3, False)
                elif jt == 14:
                    av_group(1, 4, 4, 3, 3, False)
                elif jt == 15:
                    av_group(1, 4, 4, 6, 2, False)
            for g0, glen in GROUPS:
                av_group(1, 8, 4, g0, glen, False)
            for g0, glen in GROUPS:
                av_group(1, 12, 2, g0, glen, False)
            norm_store(0)
            norm_store_pipelined()


def build_nc():
    nc = bacc.Bacc(
        "TRN2", target_bir_lowering=False, debug=False, num_devices=N_CORES
    )
    ins = [
        nc.dram_tensor("qp", [NQ, P, DC, QS], FP8, kind="ExternalInput").ap(),
        nc.dram_tensor("kp", [NQ, P, DC, QS], FP8, kind="ExternalInput").ap(),
        nc.dram_tensor("vp", [NQ, P, DC, QS], BF16, kind="ExternalInput").ap(),
        nc.dram_tensor("wq", [P, DC, H], FP8, kind="ExternalInput").ap(),
        nc.dram_tensor("bq", [P, 1], F32, kind="ExternalInput").ap(),
        nc.dram_tensor("wk", [P, DC, H], FP8, kind="ExternalInput").ap(),
        nc.dram_tensor("bk", [P, 1], F32, kind="ExternalInput").ap(),
        nc.dram_tensor("wv", [P, DC, H], BF16, kind="ExternalInput").ap(),
        nc.dram_tensor("bv", [P, H], F32, kind="ExternalInput").ap(),
    ]
    # packed [half, p, it_in_half, h]; host unpacks to [S, H]
    out_ap = nc.dram_tensor("out", [2, P, 8, H], BF16, kind="ExternalOutput").ap()
    with tile.TileContext(nc) as tc:
        _build_kernel(tc, ins, out_ap)
    nc.compile()
    return nc


_NC_CACHE = None


def _get_nc():
    global _NC_CACHE
    if _NC_CACHE is None:
        _NC_CACHE = build_nc()
    return _NC_CACHE


def _pack_xt(x_f32, dt):
    """[S, D] f32 -> X^T packed [NQ, P, DC, QS] (2-8KB DMA lines)."""
    xt = np.ascontiguousarray(x_f32.astype(dt).T)          # [D, S]
    return np.ascontiguousarray(
        xt.reshape(DC, P, NQ, QS).transpose(2, 1, 0, 3)
    )


def _pack_w(w_f32, dt, scale=1.0):
    """[D, H] f32 -> [P, DC, H] (2KB DMA lines)."""
    return np.ascontiguousarray(
        (w_f32 * scale).astype(dt).reshape(DC, P, H).transpose(1, 0, 2)
    )


def _run(inputs, trace=False, **kw):
    import ml_dtypes

    nc = _get_nc()
    bf = np.dtype(ml_dtypes.bfloat16)
    f8 = np.dtype(ml_dtypes.float8_e4m3)
    q = np.asarray(inputs["query"], dtype=np.float32)
    k = np.asarray(inputs["key"], dtype=np.float32)
    v = np.asarray(inputs["value"], dtype=np.float32)
    shared = {
        "wq": _pack_w(np.asarray(inputs["Wq"], dtype=np.float32), f8, W_SCALE),
        "wk": _pack_w(np.asarray(inputs["Wk"], dtype=np.float32), f8, W_SCALE),
        "wv": _pack_w(np.asarray(inputs["Wv"], dtype=np.float32), bf, 1.0),
        "bq": np.ascontiguousarray(
            np.asarray(inputs["bq"], dtype=np.float32).reshape(P, 1)
        ),
        "bk": np.ascontiguousarray(
            np.asarray(inputs["bk"], dtype=np.float32).reshape(P, 1)
        ),
        "bv": np.ascontiguousarray(
            np.broadcast_to(
                np.asarray(inputs["bv"], dtype=np.float32).reshape(1, H), (P, H)
            )
        ),
    }
    in_maps = [
        {
            "qp": _pack_xt(q[c], f8),
            "kp": _pack_xt(k[c], f8),
            "vp": _pack_xt(v[c], bf),
            **shared,
        }
        for c in range(N_CORES)
    ]
    res = run_bass_kernel_spmd(nc, in_maps, list(range(N_CORES)), trace=trace, **kw)
    # unpack [2, P, 8, H] -> [S, H]: s = 1024*half + 128*it + p
    out = np.stack(
        [
            res.results[c]["out"].transpose(0, 2, 1, 3).reshape(S, H)
            for c in range(N_CORES)
        ],
        axis=0,
    )
    return out.astype(np.float32), res


def kernel(**inputs) -> np.ndarray:
    out, _ = _run(inputs, trace=False)
    return out


if __name__ == "__main__":
    # smoke-build only
    build_nc()
    print("build ok")


# revision 28
# speedup vs baseline: 1.0197x; 1.0197x over previous
"""Bass/Tile kernel for a single attention head, data-parallel over B=8 on
8 TRN2 NeuronCores (one batch element per core, no collectives).

Per-core problem (S=2048, D=1024, H=128):
    q = Xq @ Wq + bq ; k = Xk @ Wk + bk ; v = Xv @ Wv + bv
    out = softmax(q k^T / sqrt(H)) v

v5 design notes (PE contracts over the partition dim):
  - X^T built on the HOST (numpy transpose + bf16 cast + repack) so the
    PE spends zero cycles transposing inputs; all DMA lines are 2-8KB.
  - Every matmul pays ~LDWEIGHTS(stat cols) + N + fixed overhead, so the
    structure minimizes instruction count and maximizes N: projections
    and scores use N=512 (the PSUM-bank max for f32), k/q/v stream in
    quarters.
  - scoresT [j, i] per j-tile; exp((1/sqrt H)x) is one ACT op per
    (j-tile, i-half) PSUM->SBUF bf16.  The ACT stream (~43us) is one of
    two walls; the schedule starts it ASAP (byte-priority q half0 + k
    first) and never lets it starve (q2/q3 projections are emitted
    INSIDE the k loop; PE is in-order).
  - v projected to natural [s, h] with NO bias: since softmax rows sum
    to 1, out = num/den + bv exactly, so bv folds into the final
    normalization (scalar_tensor_tensor: (acc*rc) + bv) for free.
  - AV keeps the fused form: stationary exp^T slice [j, i-tile], moving
    v|ones [j, 129] -> numerator AND row-sums in one accumulation.
    3 i-tiles per PSUM bank; DVE drains move 3 tiles per op.  The upper
    i-half runs in j-QUARTER phases chasing the exp i1 stream so only
    ~2us of AV trails the last exp; the lower i-half (needs only early
    i0 exp + v) fills PE slack during the exp stream.
  - Output leaves as [p, itile, h] packed halves (4KB DMA lines), host
    unpacks.  Load doorbells: weights on GpSimd queue, X on Sync
    (each dma_start costs ~680ns of issue time on its queue).
"""

import sys

if "/opt/trn_rl_repo" not in sys.path:
    sys.path.insert(0, "/opt/trn_rl_repo")

import numpy as np

import concourse.bass as bass
import concourse.tile as tile
from concourse import bacc, mybir
from concourse.bass_utils import run_bass_kernel_spmd

P = 128          # partitions
S = 2048         # sequence length (per core)
D = 1024         # input dim
H = 128          # head dim (Dq = Dk)
ST = S // P      # 16 s-tiles
DC = D // P      # 8 d-chunks
NQ = 4           # s-quarters
QS = S // NQ     # 512
N_CORES = 8

F32 = mybir.dt.float32
BF16 = mybir.dt.bfloat16
FP8 = mybir.dt.float8e4
W_SCALE = 16.0
AF = mybir.ActivationFunctionType

SOFTMAX_SCALE = 1.0 / float(np.sqrt(H))


def _build_kernel(tc, ins, out_ap):
    nc = tc.nc
    (qp, kp, vp, wq_ap, bq_ap, wk_ap, bk_ap, wv_ap, bv_ap) = ins

    with (
        tc.tile_pool(name="consts", bufs=1) as consts,
        tc.tile_pool(name="proj", bufs=1) as projp,
        tc.tile_pool(name="expp", bufs=1) as expp,
        tc.tile_pool(name="vext", bufs=1) as vexp,
        tc.tile_pool(name="accp", bufs=1) as accp,
        tc.tile_pool(name="outp", bufs=1) as outp,
        tc.tile_pool(name="xq", bufs=4) as xqp,
        tc.tile_pool(name="xk", bufs=4) as xkp,
        tc.tile_pool(name="xv", bufs=4) as xvp,
    ):
        # ---- tiny consts (no DMA) ----
        warm_a = consts.tile([P, P], BF16, tag="warm_a")
        nc.gpsimd.memset(warm_a, 0.5)
        warm_sink = nc.dram_tensor("warm_sink", [P, P], F32)

        # ---- load doorbells: weights/biases on GpSimd, X on Sync ----
        wq = consts.tile([P, DC, H], FP8, tag="wq")
        nc.gpsimd.dma_start(out=wq, in_=wq_ap)
        bq = consts.tile([P, 1], F32, tag="bq")
        nc.gpsimd.dma_start(out=bq, in_=bq_ap)
        wk = consts.tile([P, DC, H], FP8, tag="wk")
        nc.gpsimd.dma_start(out=wk, in_=wk_ap)
        bk = consts.tile([P, 1], F32, tag="bk")
        nc.gpsimd.dma_start(out=bk, in_=bk_ap)

        xq_tiles = [
            xqp.tile([P, DC, QS], FP8, tag="xq", name=f"xq{nq}")
            for nq in range(NQ)
        ]
        xk_tiles = [
            xkp.tile([P, DC, QS], FP8, tag="xk", name=f"xk{t}")
            for t in range(NQ)
        ]
        xv_tiles = [
            xvp.tile([P, DC, QS], BF16, tag="xv", name=f"xv{nq}")
            for nq in range(NQ)
        ]
        # byte-priority: q half0 + k feed the exp stream, v is last
        nc.sync.dma_start(out=xq_tiles[0], in_=qp[0])
        nc.sync.dma_start(out=xk_tiles[0], in_=kp[0])
        nc.sync.dma_start(out=xq_tiles[1], in_=qp[1])
        for t in range(1, NQ):
            nc.sync.dma_start(out=xk_tiles[t], in_=kp[t])
        nc.sync.dma_start(out=xq_tiles[2], in_=qp[2])
        nc.sync.dma_start(out=xq_tiles[3], in_=qp[3])
        for t in range(NQ):
            nc.sync.dma_start(out=xv_tiles[t], in_=vp[t])

        # wv/bvr are needed late; their doorbells ride AFTER the
        # critical q/k prefix so their bytes don't compete with it
        wv = consts.tile([P, DC, H], BF16, tag="wv")
        nc.gpsimd.dma_start(out=wv, in_=wv_ap)
        bvr = consts.tile([P, H], F32, tag="bvr")
        nc.gpsimd.dma_start(out=bvr, in_=bv_ap)
        # preload the ACT exp table set (~2.7us) during DMA dead time
        dummy = consts.tile([P, 1], F32, tag="dummy")
        nc.gpsimd.memset(dummy, 0.0)
        exp_sink = consts.tile([P, 1], BF16, tag="exp_sink")
        nc.scalar.activation(exp_sink, dummy, AF.Exp, bias=0.0, scale=1.0)

        # ---- persistent SBUF tiles ----
        qTq = [
            projp.tile([P, QS], BF16, tag=f"qT{i}", name=f"qT{i}")
            for i in range(NQ)
        ]
        kTq = [
            projp.tile([P, QS], BF16, tag=f"kT{i}", name=f"kT{i}")
            for i in range(NQ)
        ]
        ex = [
            [
                expp.tile([P, 4, 1024], BF16, tag=f"ex{h}{jq}", name=f"ex{h}{jq}")
                for jq in range(NQ)
            ]
            for h in range(2)
        ]
        vx = [
            vexp.tile([P, 4, H + 1], BF16, tag=f"vx{jq}", name=f"vx{jq}")
            for jq in range(NQ)
        ]
        for jq in range(NQ):
            nc.gpsimd.memset(vx[jq][:, :, H : H + 1], 1.0)
        acc = accp.tile([P, ST, H + 4], F32, tag="acc")
        rc_all = accp.tile([P, ST], F32, tag="rc_all")
        out_sb = [
            outp.tile([P, 8, H], BF16, tag=f"osb{hf}", name=f"osb{hf}")
            for hf in range(2)
        ]

        with (
            tc.tile_pool(name="psS", bufs=2, space="PSUM") as psS,   # 2x2 banks
            tc.tile_pool(name="psP", bufs=2, space="PSUM") as psP,   # 2x1 banks
            tc.tile_pool(name="psB", bufs=2, space="PSUM") as psB,   # 2x1 banks
        ):
            # ---- PE warm-up (HAM clock ramp needs sustained activity) ----
            ps_w = psP.tile([P, QS], F32, tag="pp", name="ps_w")
            for _ in range(90):
                nc.tensor.matmul(
                    ps_w[:, 0:P], warm_a, warm_a, start=True, stop=True
                )
            warm_sb = consts.tile([P, P], F32, tag="warm_sb")
            nc.vector.tensor_copy(warm_sb, ps_w[:, 0:P])
            nc.sync.dma_start(out=warm_sink[:, :], in_=warm_sb)

            def pe_keepalive(n):
                for _ in range(n):
                    nc.tensor.matmul(
                        ps_w[:, 0:P], warm_a, warm_a, start=True, stop=True
                    )

            def proj_quarter(xt, w, b, dst):
                ps = psP.tile([P, QS], F32, tag="pp")
                for dc in range(DC):
                    nc.tensor.matmul(
                        ps,
                        w[:, dc, :],
                        xt[:, dc, :],
                        start=(dc == 0),
                        stop=(dc == DC - 1),
                    )
                # fp8 weights ride x16 scaled (half of W is e4m3-subnormal
                # otherwise); fold the 1/16 into the bias drain
                nc.vector.tensor_scalar(
                    dst, ps, 1.0 / W_SCALE, b,
                    mybir.AluOpType.mult, mybir.AluOpType.add,
                )

            def scores_exp_split(jt):
                """scores+exp for (jt, i0) in two i-quarter ACT ops so
                the stream starts before q quarter 1 has even arrived."""
                kt_sl = kTq[0][:, (jt % 4) * P : (jt % 4 + 1) * P]
                pss = psS.tile([P, 1024], F32, tag="ps", name=f"pss_sp{jt}")
                nc.tensor.matmul(
                    pss[:, 0:QS], kt_sl, qTq[0], start=True, stop=True
                )
                nc.scalar.activation(
                    ex[0][0][:, jt, 0:QS],
                    pss[:, 0:QS],
                    AF.Exp,
                    bias=0.0,
                    scale=SOFTMAX_SCALE,
                )
                return pss

            def scores_exp_split2(jt, pss):
                nc.tensor.matmul(
                    pss[:, QS:1024], kTq[0][:, (jt % 4) * P : (jt % 4 + 1) * P],
                    qTq[1], start=True, stop=True
                )
                nc.scalar.activation(
                    ex[0][0][:, jt, QS:1024],
                    pss[:, QS:1024],
                    AF.Exp,
                    bias=0.0,
                    scale=SOFTMAX_SCALE,
                )

            def scores_exp(jt, hf):
                kt_sl = kTq[jt // 4][:, (jt % 4) * P : (jt % 4 + 1) * P]
                pss = psS.tile([P, 1024], F32, tag="ps")
                for nb in range(2):
                    nc.tensor.matmul(
                        pss[:, nb * QS : (nb + 1) * QS],
                        kt_sl,
                        qTq[2 * hf + nb],
                        start=True,
                        stop=True,
                    )
                nc.scalar.activation(
                    ex[hf][jt // 4][:, jt % 4, :],
                    pss,
                    AF.Exp,
                    bias=0.0,
                    scale=SOFTMAX_SCALE,
                )

            # v quarter projection split into s-tile emission chunks so
            # it can fill PE slack between ACT-paced scores
            vps = {}

            def vproj_stile(jq, st):
                if jq not in vps:
                    vps[jq] = psP.tile([P, QS], F32, tag="pp", name=f"vps{jq}")
                ps = vps[jq]
                for dc in range(DC):
                    nc.tensor.matmul(
                        ps[:, st * P : (st + 1) * P],
                        xv_tiles[jq][:, dc, st * P : (st + 1) * P],
                        wv[:, dc, :],
                        start=(dc == 0),
                        stop=(dc == DC - 1),
                    )

            def vdrain(jq):
                nc.vector.tensor_copy(
                    vx[jq][:, :, 0:H],
                    vps[jq].rearrange("p (a b) -> p a b", b=P),
                )

            GROUPS = [(0, 3), (3, 3), (6, 2)]

            def av_group(ihalf, j0, nj, g0, glen, first):
                """AV partials: i-tiles [8ihalf+g0, +glen) x j-tiles
                [j0, j0+nj), 3 i-tiles per PSUM bank, one DVE drain."""
                i0 = 8 * ihalf
                po = psB.tile([P, 3, H + 4], F32, tag="po")
                for m in range(glen):
                    k = g0 + m
                    for dj in range(nj):
                        jt = j0 + dj
                        nc.tensor.matmul(
                            po[:, m, 0 : H + 1],
                            ex[ihalf][jt // 4][:, jt % 4, k * P : (k + 1) * P],
                            vx[jt // 4][:, jt % 4, :],
                            start=(dj == 0),
                            stop=(dj == nj - 1),
                        )
                dst = acc[:, i0 + g0 : i0 + g0 + glen, :]
                src = po[:, 0:glen, :]
                if first:
                    nc.vector.tensor_copy(dst, src)
                else:
                    nc.vector.tensor_add(dst, dst, src)

            def norm_store_pipelined():
                """Only j14-15 trails the final exp; per group: DVE
                drain-add + batched recip, scale-muls on the (now idle)
                ACT engine, bv-add + store on DVE."""
                for g0, glen in GROUPS:
                    av_group(1, 14, 2, g0, glen, False)
                    a0 = 8 + g0
                    nc.vector.reciprocal(
                        rc_all[:, a0 : a0 + glen],
                        acc[:, a0 : a0 + glen, H : H + 1].squeeze(-1),
                    )
                    dst = out_sb[1][:, g0 : g0 + glen, :]
                    for m in range(glen):
                        nc.scalar.activation(
                            out_sb[1][:, g0 + m, :],
                            acc[:, a0 + m, 0:H],
                            AF.Copy,
                            bias=0.0,
                            scale=rc_all[:, a0 + m : a0 + m + 1],
                        )
                    bv_bc = bvr[:, :].unsqueeze(1).broadcast_to([P, glen, H])
                    nc.vector.tensor_add(dst, dst, bv_bc)
                    nc.sync.dma_start(
                        out=out_ap[1, :, g0 : g0 + glen, :], in_=dst
                    )

            def norm_store(ihalf):
                """Batched reciprocal; out = acc*rc + bv (bv folds in
                free since softmax rows sum to 1); one packed half DMA."""
                i0 = 8 * ihalf
                nc.vector.reciprocal(
                    rc_all[:, i0 : i0 + 8],
                    acc[:, i0 : i0 + 8, H : H + 1].squeeze(-1),
                )
                for g0, glen in GROUPS:
                    rc_bc = (
                        rc_all[:, i0 + g0 : i0 + g0 + glen]
                        .unsqueeze(-1)
                        .broadcast_to([P, glen, H])
                    )
                    dst = out_sb[ihalf][:, g0 : g0 + glen, :]
                    nc.vector.tensor_mul(
                        dst, acc[:, i0 + g0 : i0 + g0 + glen, 0:H], rc_bc
                    )
                    bv_bc = bvr[:, :].unsqueeze(1).broadcast_to(
                        [P, glen, H]
                    )
                    nc.vector.tensor_add(dst, dst, bv_bc)
                    nc.sync.dma_start(
                        out=out_ap[ihalf, :, g0 : g0 + glen, :], in_=dst
                    )

            # ---- emission order == intended engine execution order ----
            # PE is in-order: every insertion is placed at the point
            # where its data has just arrived, sized ~<=2us so the
            # ACT-paced scores stream never starves for long.
            proj_quarter(xq_tiles[0], wq, bq, qTq[0])
            pe_keepalive(20)
            proj_quarter(xk_tiles[0], wk, bk, kTq[0])
            ps_j0 = scores_exp_split(0)
            ps_j1 = scores_exp_split(1)
            proj_quarter(xq_tiles[1], wq, bq, qTq[1])
            scores_exp_split2(0, ps_j0)
            scores_exp_split2(1, ps_j1)
            ps_j2 = scores_exp_split(2)
            scores_exp_split2(2, ps_j2)
            ps_j3 = scores_exp_split(3)
            scores_exp_split2(3, ps_j3)
            for kq in range(1, NQ):
                proj_quarter(xk_tiles[kq], wk, bk, kTq[kq])
                for jt in range(4 * kq, 4 * kq + 4):
                    scores_exp(jt, 0)
                    if jt == 12:
                        vproj_stile(0, 0)
                        vproj_stile(0, 1)
                    elif jt == 13:
                        proj_quarter(xq_tiles[2], wq, bq, qTq[2])
                    elif jt == 14:
                        vproj_stile(0, 2)
                        vproj_stile(0, 3)
                        vdrain(0)
                        av_group(0, 0, 4, 0, 3, True)
                    elif jt == 15:
                        proj_quarter(xq_tiles[3], wq, bq, qTq[3])
                        av_group(0, 0, 4, 3, 3, True)

            for jt in range(ST):
                scores_exp(jt, 1)
                if jt == 0:
                    av_group(0, 0, 4, 6, 2, True)
                    vproj_stile(1, 0)
                    vproj_stile(1, 1)
                elif jt == 1:
                    vproj_stile(1, 2)
                    vproj_stile(1, 3)
                    vdrain(1)
                elif jt == 2:
                    vproj_stile(2, 0)
                    vproj_stile(2, 1)
                    vproj_stile(2, 2)
                    vproj_stile(2, 3)
                    vdrain(2)
                elif jt == 3:
                    av_group(0, 4, 4, 0, 3, False)
                elif jt == 4:
                    av_group(0, 4, 4, 3, 3, False)
                elif jt == 5:
                    av_group(0, 4, 4, 6, 2, False)
                elif jt == 6:
                    vproj_stile(3, 0)
                    vproj_stile(3, 1)
                elif jt == 7:
                    vproj_stile(3, 2)
                    vproj_stile(3, 3)
                    vdrain(3)
                elif jt == 8:
                    av_group(0, 8, 8, 0, 3, False)
                elif jt == 9:
                    av_group(0, 8, 8, 3, 3, False)
                elif jt == 10:
                    av_group(0, 8, 8, 6, 2, False)
                elif jt == 11:
                    av_group(1, 0, 4, 0, 3, True)
                elif jt == 12:
                    av_group(1, 0, 4, 3, 3, True)
                elif jt == 13:
                    av_group(1, 0, 4, 6, 2, True)
                    av_group(1, 4, 4, 0, 3, False)
                elif jt == 14:
                    av_group(1, 4, 4, 3, 3, False)
                elif jt == 15:
                    av_group(1, 4, 4, 6, 2, False)
            for g0, glen in GROUPS:
                av_group(1, 8, 4, g0, glen, False)
            for g0, glen in GROUPS:
                av_group(1, 12, 2, g0, glen, False)
            norm_store(0)
            norm_store_pipelined()


def build_nc():
    nc = bacc.Bacc(
        "TRN2", target_bir_lowering=False, debug=False, num_devices=N_CORES
    )
    ins = [
        nc.dram_tensor("qp", [NQ, P, DC, QS], FP8, kind="ExternalInput").ap(),
        nc.dram_tensor("kp", [NQ, P, DC, QS], FP8, kind="ExternalInput").ap(),
        nc.dram_tensor("vp", [NQ, P, DC, QS], BF16, kind="ExternalInput").ap(),
        nc.dram_tensor("wq", [P, DC, H], FP8, kind="ExternalInput").ap(),
        nc.dram_tensor("bq", [P, 1], F32, kind="ExternalInput").ap(),
        nc.dram_tensor("wk", [P, DC, H], FP8, kind="ExternalInput").ap(),
        nc.dram_tensor("bk", [P, 1], F32, kind="ExternalInput").ap(),
        nc.dram_tensor("wv", [P, DC, H], BF16, kind="ExternalInput").ap(),
        nc.dram_tensor("bv", [P, H], F32, kind="ExternalInput").ap(),
    ]
    # packed [half, p, it_in_half, h]; host unpacks to [S, H]
    out_ap = nc.dram_tensor("out", [2, P, 8, H], BF16, kind="ExternalOutput").ap()
    with tile.TileContext(nc) as tc:
        _build_kernel(tc, ins, out_ap)
    nc.compile()
    return nc


_NC_CACHE = None


def _get_nc():
    global _NC_CACHE
    if _NC_CACHE is None:
        _NC_CACHE = build_nc()
    return _NC_CACHE


def _pack_xt(x_f32, dt):
    """[S, D] f32 -> X^T packed [NQ, P, DC, QS] (2-8KB DMA lines)."""
    xt = np.ascontiguousarray(x_f32.astype(dt).T)          # [D, S]
    return np.ascontiguousarray(
        xt.reshape(DC, P, NQ, QS).transpose(2, 1, 0, 3)
    )


def _pack_w(w_f32, dt, scale=1.0):
    """[D, H] f32 -> [P, DC, H] (2KB DMA lines)."""
    return np.ascontiguousarray(
        (w_f32 * scale).astype(dt).reshape(DC, P, H).transpose(1, 0, 2)
    )


def _run(inputs, trace=False, **kw):
    import ml_dtypes

    nc = _get_nc()
    bf = np.dtype(ml_dtypes.bfloat16)
    f8 = np.dtype(ml_dtypes.float8_e4m3)
    q = np.asarray(inputs["query"], dtype=np.float32)
    k = np.asarray(inputs["key"], dtype=np.float32)
    v = np.asarray(inputs["value"], dtype=np.float32)
    shared = {
        "wq": _pack_w(np.asarray(inputs["Wq"], dtype=np.float32), f8, W_SCALE),
        "wk": _pack_w(np.asarray(inputs["Wk"], dtype=np.float32), f8, W_SCALE),
        "wv": _pack_w(np.asarray(inputs["Wv"], dtype=np.float32), bf, 1.0),
        "bq": np.ascontiguousarray(
            np.asarray(inputs["bq"], dtype=np.float32).reshape(P, 1)
        ),
        "bk": np.ascontiguousarray(
            np.asarray(inputs["bk"], dtype=np.float32).reshape(P, 1)
        ),
        "bv": np.ascontiguousarray(
            np.broadcast_to(
                np.asarray(inputs["bv"], dtype=np.float32).reshape(1, H), (P, H)
            )
        ),
    }
    in_maps = [
        {
            "qp": _pack_xt(q[c], f8),
            "kp": _pack_xt(k[c], f8),
            "vp": _pack_xt(v[c], bf),
            **shared,
        }
        for c in range(N_CORES)
    ]
    res = run_bass_kernel_spmd(nc, in_maps, list(range(N_CORES)), trace=trace, **kw)
    # unpack [2, P, 8, H] -> [S, H]: s = 1024*half + 128*it + p
    out = np.stack(
        [
            res.results[c]["out"].transpose(0, 2, 1, 3).reshape(S, H)
            for c in range(N_CORES)
        ],
        axis=0,
    )
    return out.astype(np.float32), res


def kernel(**inputs) -> np.ndarray:
    out, _ = _run(inputs, trace=False)
    return out


if __name__ == "__main__":
    # smoke-build only
    build_nc()
    print("build ok")


# revision 29
# speedup vs baseline: 1.2348x; 1.2109x over previous
"""Bass/Tile kernel for a single attention head, data-parallel over B=8 on
8 TRN2 NeuronCores (one batch element per core, no collectives).

Per-core problem (S=2048, D=1024, H=128):
    q = Xq @ Wq + bq ; k = Xk @ Wk + bk ; v = Xv @ Wv + bv
    out = softmax(q k^T / sqrt(H)) v

v5 design notes (PE contracts over the partition dim):
  - X^T built on the HOST (numpy transpose + bf16 cast + repack) so the
    PE spends zero cycles transposing inputs; all DMA lines are 2-8KB.
  - Every matmul pays ~LDWEIGHTS(stat cols) + N + fixed overhead, so the
    structure minimizes instruction count and maximizes N: projections
    and scores use N=512 (the PSUM-bank max for f32), k/q/v stream in
    quarters.
  - scoresT [j, i] per j-tile; exp((1/sqrt H)x) is one ACT op per
    (j-tile, i-half) PSUM->SBUF bf16.  The ACT stream (~43us) is one of
    two walls; the schedule starts it ASAP (byte-priority q half0 + k
    first) and never lets it starve (q2/q3 projections are emitted
    INSIDE the k loop; PE is in-order).
  - v projected to natural [s, h] with NO bias: since softmax rows sum
    to 1, out = num/den + bv exactly, so bv folds into the final
    normalization (scalar_tensor_tensor: (acc*rc) + bv) for free.
  - AV keeps the fused form: stationary exp^T slice [j, i-tile], moving
    v|ones [j, 129] -> numerator AND row-sums in one accumulation.
    3 i-tiles per PSUM bank; DVE drains move 3 tiles per op.  The upper
    i-half runs in j-QUARTER phases chasing the exp i1 stream so only
    ~2us of AV trails the last exp; the lower i-half (needs only early
    i0 exp + v) fills PE slack during the exp stream.
  - Output leaves as [p, itile, h] packed halves (4KB DMA lines), host
    unpacks.  Load doorbells: weights on GpSimd queue, X on Sync
    (each dma_start costs ~680ns of issue time on its queue).
"""

import sys

if "/opt/trn_rl_repo" not in sys.path:
    sys.path.insert(0, "/opt/trn_rl_repo")

import numpy as np

import concourse.bass as bass
import concourse.tile as tile
from concourse import bacc, mybir
from concourse.bass_utils import run_bass_kernel_spmd

P = 128          # partitions
S = 2048         # sequence length (per core)
D = 1024         # input dim
H = 128          # head dim (Dq = Dk)
ST = S // P      # 16 s-tiles
DC = D // P      # 8 d-chunks
NQ = 4           # s-quarters
QS = S // NQ     # 512
N_CORES = 8

F32 = mybir.dt.float32
BF16 = mybir.dt.bfloat16
FP8 = mybir.dt.float8e4
W_SCALE = 16.0
AF = mybir.ActivationFunctionType

SOFTMAX_SCALE = 1.0 / float(np.sqrt(H))


def _build_kernel(tc, ins, out_ap):
    nc = tc.nc
    (qp, kp, vp, wq_ap, bq_ap, wk_ap, bk_ap, wv_ap, bv_ap) = ins

    with (
        tc.tile_pool(name="consts", bufs=1) as consts,
        tc.tile_pool(name="proj", bufs=1) as projp,
        tc.tile_pool(name="expp", bufs=1) as expp,
        tc.tile_pool(name="vext", bufs=1) as vexp,
        tc.tile_pool(name="accp", bufs=1) as accp,
        tc.tile_pool(name="outp", bufs=1) as outp,
        tc.tile_pool(name="xq", bufs=4) as xqp,
        tc.tile_pool(name="xk", bufs=4) as xkp,
        tc.tile_pool(name="xv", bufs=4) as xvp,
    ):
        # ---- tiny consts (no DMA) ----
        warm_a = consts.tile([P, P], BF16, tag="warm_a")
        nc.gpsimd.memset(warm_a, 0.5)
        warm_sink = nc.dram_tensor("warm_sink", [P, P], F32)

        # ---- load doorbells: weights/biases on GpSimd, X on Sync ----
        wq = consts.tile([P, DC, H], FP8, tag="wq")
        nc.gpsimd.dma_start(out=wq, in_=wq_ap)
        bq = consts.tile([P, 1], F32, tag="bq")
        nc.gpsimd.dma_start(out=bq, in_=bq_ap)
        wk = consts.tile([P, DC, H], FP8, tag="wk")
        nc.gpsimd.dma_start(out=wk, in_=wk_ap)
        bk = consts.tile([P, 1], F32, tag="bk")
        nc.gpsimd.dma_start(out=bk, in_=bk_ap)

        xq_tiles = [
            xqp.tile([P, DC, QS], FP8, tag="xq", name=f"xq{nq}")
            for nq in range(NQ)
        ]
        xk_tiles = [
            xkp.tile([P, DC, QS], FP8, tag="xk", name=f"xk{t}")
            for t in range(NQ)
        ]
        xv_tiles = [
            xvp.tile([P, DC, QS], BF16, tag="xv", name=f"xv{nq}")
            for nq in range(NQ)
        ]
        # byte-priority: q half0 + k feed the exp stream, v is last
        nc.sync.dma_start(out=xq_tiles[0], in_=qp[0])
        nc.sync.dma_start(out=xk_tiles[0], in_=kp[0])
        nc.sync.dma_start(out=xq_tiles[1], in_=qp[1])
        for t in range(1, NQ):
            nc.sync.dma_start(out=xk_tiles[t], in_=kp[t])
        nc.sync.dma_start(out=xq_tiles[2], in_=qp[2])
        nc.sync.dma_start(out=xq_tiles[3], in_=qp[3])
        for t in range(NQ):
            nc.sync.dma_start(out=xv_tiles[t], in_=vp[t])

        # wv/bvr are needed late; their doorbells ride AFTER the
        # critical q/k prefix so their bytes don't compete with it
        wv = consts.tile([P, DC, H], BF16, tag="wv")
        nc.gpsimd.dma_start(out=wv, in_=wv_ap)
        bvr = consts.tile([P, H], F32, tag="bvr")
        nc.gpsimd.dma_start(out=bvr, in_=bv_ap)
        # preload the ACT exp table set (~2.7us) during DMA dead time
        dummy = consts.tile([P, 1], F32, tag="dummy")
        nc.gpsimd.memset(dummy, 0.0)
        exp_sink = consts.tile([P, 1], BF16, tag="exp_sink")
        nc.scalar.activation(exp_sink, dummy, AF.Exp, bias=0.0, scale=1.0)

        # ---- persistent SBUF tiles ----
        qTq = [
            projp.tile([P, QS], BF16, tag=f"qT{i}", name=f"qT{i}")
            for i in range(NQ)
        ]
        kTq = [
            projp.tile([P, QS], BF16, tag=f"kT{i}", name=f"kT{i}")
            for i in range(NQ)
        ]
        ex = [
            [
                expp.tile([P, 4, 1024], BF16, tag=f"ex{h}{jq}", name=f"ex{h}{jq}")
                for jq in range(NQ)
            ]
            for h in range(2)
        ]
        vx = [
            vexp.tile([P, 4, H + 1], BF16, tag=f"vx{jq}", name=f"vx{jq}")
            for jq in range(NQ)
        ]
        for jq in range(NQ):
            nc.gpsimd.memset(vx[jq][:, :, H : H + 1], 1.0)
        acc = accp.tile([P, ST, H + 4], F32, tag="acc")
        rc_all = accp.tile([P, ST], F32, tag="rc_all")
        out_sb = [
            outp.tile([P, 8, H], BF16, tag=f"osb{hf}", name=f"osb{hf}")
            for hf in range(2)
        ]

        with (
            tc.tile_pool(name="psS", bufs=2, space="PSUM") as psS,   # 2x2 banks
            tc.tile_pool(name="psP", bufs=2, space="PSUM") as psP,   # 2x1 banks
            tc.tile_pool(name="psB", bufs=2, space="PSUM") as psB,   # 2x1 banks
        ):
            # ---- PE warm-up (HAM clock ramp needs sustained activity) ----
            ps_w = psP.tile([P, QS], F32, tag="pp", name="ps_w")
            for _ in range(90):
                nc.tensor.matmul(
                    ps_w[:, 0:P], warm_a, warm_a, start=True, stop=True
                )
            warm_sb = consts.tile([P, P], F32, tag="warm_sb")
            nc.vector.tensor_copy(warm_sb, ps_w[:, 0:P])
            nc.sync.dma_start(out=warm_sink[:, :], in_=warm_sb)

            def pe_keepalive(n):
                for _ in range(n):
                    nc.tensor.matmul(
                        ps_w[:, 0:P], warm_a, warm_a, start=True, stop=True
                    )

            def proj_quarter(xt, w, b, dst):
                ps = psP.tile([P, QS], F32, tag="pp")
                for dc in range(DC):
                    nc.tensor.matmul(
                        ps,
                        w[:, dc, :],
                        xt[:, dc, :],
                        start=(dc == 0),
                        stop=(dc == DC - 1),
                    )
                # fp8 weights ride x16 scaled (half of W is e4m3-subnormal
                # otherwise); fold the 1/16 into the bias drain
                nc.vector.tensor_scalar(
                    dst, ps, 1.0 / W_SCALE, b,
                    mybir.AluOpType.mult, mybir.AluOpType.add,
                )

            def scores_exp_split(jt):
                """scores+exp for (jt, i0) in two i-quarter ACT ops so
                the stream starts before q quarter 1 has even arrived."""
                kt_sl = kTq[0][:, (jt % 4) * P : (jt % 4 + 1) * P]
                pss = psS.tile([P, 1024], F32, tag="ps", name=f"pss_sp{jt}")
                nc.tensor.matmul(
                    pss[:, 0:QS], kt_sl, qTq[0], start=True, stop=True
                )
                nc.scalar.activation(
                    ex[0][0][:, jt, 0:QS],
                    pss[:, 0:QS],
                    AF.Exp,
                    bias=0.0,
                    scale=SOFTMAX_SCALE,
                )
                return pss

            def scores_exp_split2(jt, pss):
                nc.tensor.matmul(
                    pss[:, QS:1024], kTq[0][:, (jt % 4) * P : (jt % 4 + 1) * P],
                    qTq[1], start=True, stop=True
                )
                nc.scalar.activation(
                    ex[0][0][:, jt, QS:1024],
                    pss[:, QS:1024],
                    AF.Exp,
                    bias=0.0,
                    scale=SOFTMAX_SCALE,
                )

            def scores_exp(jt, hf):
                kt_sl = kTq[jt // 4][:, (jt % 4) * P : (jt % 4 + 1) * P]
                pss = psS.tile([P, 1024], F32, tag="ps")
                for nb in range(2):
                    nc.tensor.matmul(
                        pss[:, nb * QS : (nb + 1) * QS],
                        kt_sl,
                        qTq[2 * hf + nb],
                        start=True,
                        stop=True,
                    )
                nc.scalar.activation(
                    ex[hf][jt // 4][:, jt % 4, :],
                    pss,
                    AF.Exp,
                    bias=0.0,
                    scale=SOFTMAX_SCALE,
                )

            # v quarter projection split into s-tile emission chunks so
            # it can fill PE slack between ACT-paced scores
            vps = {}

            def vproj_stile(jq, st):
                if jq not in vps:
                    vps[jq] = psP.tile([P, QS], F32, tag="pp", name=f"vps{jq}")
                ps = vps[jq]
                for dc in range(DC):
                    nc.tensor.matmul(
                        ps[:, st * P : (st + 1) * P],
                        xv_tiles[jq][:, dc, st * P : (st + 1) * P],
                        wv[:, dc, :],
                        start=(dc == 0),
                        stop=(dc == DC - 1),
                    )

            def vdrain(jq):
                nc.vector.tensor_copy(
                    vx[jq][:, :, 0:H],
                    vps[jq].rearrange("p (a b) -> p a b", b=P),
                )

            GROUPS = [(0, 3), (3, 3), (6, 2)]

            def av_group(ihalf, j0, nj, g0, glen, first):
                """AV partials: i-tiles [8ihalf+g0, +glen) x j-tiles
                [j0, j0+nj), 3 i-tiles per PSUM bank, one DVE drain."""
                i0 = 8 * ihalf
                po = psB.tile([P, 3, H + 4], F32, tag="po")
                for m in range(glen):
                    k = g0 + m
                    for dj in range(nj):
                        jt = j0 + dj
                        nc.tensor.matmul(
                            po[:, m, 0 : H + 1],
                            ex[ihalf][jt // 4][:, jt % 4, k * P : (k + 1) * P],
                            vx[jt // 4][:, jt % 4, :],
                            start=(dj == 0),
                            stop=(dj == nj - 1),
                        )
                dst = acc[:, i0 + g0 : i0 + g0 + glen, :]
                src = po[:, 0:glen, :]
                if first:
                    nc.vector.tensor_copy(dst, src)
                else:
                    nc.vector.tensor_add(dst, dst, src)

            def norm_store_pipelined():
                """Final j-quarter AV for the upper half with per-group
                recip/scale/store chained right behind each drain."""
                for g0, glen in GROUPS:
                    av_group(1, 12, 4, g0, glen, False)
                    a0 = 8 + g0
                    nc.vector.reciprocal(
                        rc_all[:, a0 : a0 + glen],
                        acc[:, a0 : a0 + glen, H : H + 1].squeeze(-1),
                    )
                    rc_bc = (
                        rc_all[:, a0 : a0 + glen]
                        .unsqueeze(-1)
                        .broadcast_to([P, glen, H])
                    )
                    dst = out_sb[1][:, g0 : g0 + glen, :]
                    nc.vector.tensor_mul(dst, acc[:, a0 : a0 + glen, 0:H], rc_bc)
                    bv_bc = bvr[:, :].unsqueeze(1).broadcast_to([P, glen, H])
                    nc.vector.tensor_add(dst, dst, bv_bc)
                    nc.sync.dma_start(
                        out=out_ap[1, :, g0 : g0 + glen, :], in_=dst
                    )

            def norm_store(ihalf):
                """Batched reciprocal; out = acc*rc + bv (bv folds in
                free since softmax rows sum to 1); one packed half DMA."""
                i0 = 8 * ihalf
                nc.vector.reciprocal(
                    rc_all[:, i0 : i0 + 8],
                    acc[:, i0 : i0 + 8, H : H + 1].squeeze(-1),
                )
                for g0, glen in GROUPS:
                    rc_bc = (
                        rc_all[:, i0 + g0 : i0 + g0 + glen]
                        .unsqueeze(-1)
                        .broadcast_to([P, glen, H])
                    )
                    dst = out_sb[ihalf][:, g0 : g0 + glen, :]
                    nc.vector.tensor_mul(
                        dst, acc[:, i0 + g0 : i0 + g0 + glen, 0:H], rc_bc
                    )
                    bv_bc = bvr[:, :].unsqueeze(1).broadcast_to(
                        [P, glen, H]
                    )
                    nc.vector.tensor_add(dst, dst, bv_bc)
                    nc.sync.dma_start(
                        out=out_ap[ihalf, :, g0 : g0 + glen, :], in_=dst
                    )

            # ---- emission order == intended engine execution order ----
            # PE is in-order: every insertion is placed at the point
            # where its data has just arrived, sized ~<=2us so the
            # ACT-paced scores stream never starves for long.
            proj_quarter(xq_tiles[0], wq, bq, qTq[0])
            pe_keepalive(20)
            proj_quarter(xk_tiles[0], wk, bk, kTq[0])
            ps_j0 = scores_exp_split(0)
            ps_j1 = scores_exp_split(1)
            proj_quarter(xq_tiles[1], wq, bq, qTq[1])
            scores_exp_split2(0, ps_j0)
            scores_exp_split2(1, ps_j1)
            ps_j2 = scores_exp_split(2)
            scores_exp_split2(2, ps_j2)
            ps_j3 = scores_exp_split(3)
            scores_exp_split2(3, ps_j3)
            for kq in range(1, NQ):
                proj_quarter(xk_tiles[kq], wk, bk, kTq[kq])
                for jt in range(4 * kq, 4 * kq + 4):
                    scores_exp(jt, 0)
                    if jt == 12:
                        vproj_stile(0, 0)
                        vproj_stile(0, 1)
                    elif jt == 13:
                        proj_quarter(xq_tiles[2], wq, bq, qTq[2])
                    elif jt == 14:
                        vproj_stile(0, 2)
                        vproj_stile(0, 3)
                        vdrain(0)
                        av_group(0, 0, 4, 0, 3, True)
                    elif jt == 15:
                        proj_quarter(xq_tiles[3], wq, bq, qTq[3])
                        av_group(0, 0, 4, 3, 3, True)

            for jt in range(ST):
                scores_exp(jt, 1)
                if jt == 0:
                    av_group(0, 0, 4, 6, 2, True)
                    vproj_stile(1, 0)
                    vproj_stile(1, 1)
                elif jt == 1:
                    vproj_stile(1, 2)
                    vproj_stile(1, 3)
                    vdrain(1)
                elif jt == 2:
                    vproj_stile(2, 0)
                    vproj_stile(2, 1)
                    vproj_stile(2, 2)
                    vproj_stile(2, 3)
                    vdrain(2)
                elif jt == 3:
                    av_group(0, 4, 4, 0, 3, False)
                elif jt == 4:
                    av_group(0, 4, 4, 3, 3, False)
                elif jt == 5:
                    av_group(0, 4, 4, 6, 2, False)
                elif jt == 6:
                    vproj_stile(3, 0)
                    vproj_stile(3, 1)
                elif jt == 7:
                    vproj_stile(3, 2)
                    vproj_stile(3, 3)
                    vdrain(3)
                elif jt == 8:
                    av_group(0, 8, 8, 0, 3, False)
                elif jt == 9:
                    av_group(0, 8, 8, 3, 3, False)
                elif jt == 10:
                    av_group(0, 8, 8, 6, 2, False)
                elif jt == 11:
                    av_group(1, 0, 4, 0, 3, True)
                elif jt == 12:
                    av_group(1, 0, 4, 3, 3, True)
                elif jt == 13:
                    av_group(1, 0, 4, 6, 2, True)
                    av_group(1, 4, 4, 0, 3, False)
                elif jt == 14:
                    av_group(1, 4, 4, 3, 3, False)
                elif jt == 15:
                    av_group(1, 4, 4, 6, 2, False)
            for g0, glen in GROUPS:
                av_group(1, 8, 4, g0, glen, False)
            norm_store(0)
            norm_store_pipelined()


def build_nc():
    nc = bacc.Bacc(
        "TRN2", target_bir_lowering=False, debug=False, num_devices=N_CORES
    )
    ins = [
        nc.dram_tensor("qp", [NQ, P, DC, QS], FP8, kind="ExternalInput").ap(),
        nc.dram_tensor("kp", [NQ, P, DC, QS], FP8, kind="ExternalInput").ap(),
        nc.dram_tensor("vp", [NQ, P, DC, QS], BF16, kind="ExternalInput").ap(),
        nc.dram_tensor("wq", [P, DC, H], FP8, kind="ExternalInput").ap(),
        nc.dram_tensor("bq", [P, 1], F32, kind="ExternalInput").ap(),
        nc.dram_tensor("wk", [P, DC, H], FP8, kind="ExternalInput").ap(),
        nc.dram_tensor("bk", [P, 1], F32, kind="ExternalInput").ap(),
        nc.dram_tensor("wv", [P, DC, H], BF16, kind="ExternalInput").ap(),
        nc.dram_tensor("bv", [P, H], F32, kind="ExternalInput").ap(),
    ]
    # packed [half, p, it_in_half, h]; host unpacks to [S, H]
    out_ap = nc.dram_tensor("out", [2, P, 8, H], BF16, kind="ExternalOutput").ap()
    with tile.TileContext(nc) as tc:
        _build_kernel(tc, ins, out_ap)
    nc.compile()
    return nc


_NC_CACHE = None


def _get_nc():
    global _NC_CACHE
    if _NC_CACHE is None:
        _NC_CACHE = build_nc()
    return _NC_CACHE


def _pack_xt(x_f32, dt):
    """[S, D] f32 -> X^T packed [NQ, P, DC, QS] (2-8KB DMA lines)."""
    xt = np.ascontiguousarray(x_f32.astype(dt).T)          # [D, S]
    return np.ascontiguousarray(
        xt.reshape(DC, P, NQ, QS).transpose(2, 1, 0, 3)
    )


def _pack_w(w_f32, dt, scale=1.0):
    """[D, H] f32 -> [P, DC, H] (2KB DMA lines)."""
    return np.ascontiguousarray(
        (w_f32 * scale).astype(dt).reshape(DC, P, H).transpose(1, 0, 2)
    )


def _run(inputs, trace=False, **kw):
    import ml_dtypes

    nc = _get_nc()
    bf = np.dtype(ml_dtypes.bfloat16)
    f8 = np.dtype(ml_dtypes.float8_e4m3)
    q = np.asarray(inputs["query"], dtype=np.float32)
    k = np.asarray(inputs["key"], dtype=np.float32)
    v = np.asarray(inputs["value"], dtype=np.float32)
    shared = {
        "wq": _pack_w(np.asarray(inputs["Wq"], dtype=np.float32), f8, W_SCALE),
        "wk": _pack_w(np.asarray(inputs["Wk"], dtype=np.float32), f8, W_SCALE),
        "wv": _pack_w(np.asarray(inputs["Wv"], dtype=np.float32), bf, 1.0),
        "bq": np.ascontiguousarray(
            np.asarray(inputs["bq"], dtype=np.float32).reshape(P, 1)
        ),
        "bk": np.ascontiguousarray(
            np.asarray(inputs["bk"], dtype=np.float32).reshape(P, 1)
        ),
        "bv": np.ascontiguousarray(
            np.broadcast_to(
                np.asarray(inputs["bv"], dtype=np.float32).reshape(1, H), (P, H)
            )
        ),
    }
    in_maps = [
        {
            "qp": _pack_xt(q[c], f8),
            "kp": _pack_xt(k[c], f8),
            "vp": _pack_xt(v[c], bf),
            **shared,
        }
        for c in range(N_CORES)
    ]
    res = run_bass_kernel_spmd(nc, in_maps, list(range(N_CORES)), trace=trace, **kw)
    # unpack [2, P, 8, H] -> [S, H]: s = 1024*half + 128*it + p
    out = np.stack(
        [
            res.results[c]["out"].transpose(0, 2, 1, 3).reshape(S, H)
            for c in range(N_CORES)
        ],
        axis=0,
    )
    return out.astype(np.float32), res


def kernel(**inputs) -> np.ndarray:
    out, _ = _run(inputs, trace=False)
    return out


if __name__ == "__main__":
    # smoke-build only
    build_nc()
    print("build ok")
